# revision 1
# baseline (speedup 1.0000x reference)
"""LocallyConnected2d kernel for 8 TRN2 NeuronCores (Bass/Tile).

Problem (hardcoded):
  features [32, 64, 64, 64] f32, weights [62, 62, 64, 64, 3, 3] f32,
  bias [62, 62, 64] f32 -> out [32, 64, 62, 62] f32
  out[b,o,h,w] = sum_{c,i,j} x[b,c,h+i,w+j] * W[h,w,o,c,i,j] + bias[h,w,o]

Strategy:
  - Shard over Hout: 8 cores x 8 output rows (bands [0,8,...,48,54], last two
    overlap by 2 rows; host takes the canonical rows from each core).
  - bf16 compute on the PE. Per output location (h,w): contraction over
    (c,i,j)=576 done as 5 accumulating matmuls into PSUM:
      kt r in {0,1,2}: K=128 = [c x (i=r,j=0) | c x (i=r,j=1)]
      kt 3:            K=128 = [c x (0,2)     | c x (1,2)]
      kt 4:            K=64  =  c x (2,2)
    The K=128 tiles read a host-baked "dual shifted" feature layout:
      featA[p<64]  = x[c, t, w]      featA[p>=64] = x[c, t, w+1]
      featB[p<64]  = x[c, t, w]      featB[p>=64] = x[c, t+1, w]
    so a single [128, 32] AP slice is a ready im2col patch (batch = free dim).
  - Patches are the stationary operand (cols = batch, 32); weights stream as
    the moving operand [K, 64]. 4 output rows (j=0..3) are packed into one
    PSUM tile [128, 64] via col tile_position=(0, 32j).
  - PSUM -> SBUF (bf16 cast) -> one contiguous DMA per half-band; host
    unpacks, adds bias, and assembles the full f32 output.
"""

import numpy as np
import ml_dtypes

BF16 = ml_dtypes.bfloat16

B, CIN, COUT = 32, 64, 64
H = W = 64
HOUT = WOUT = 62
NCORES = 8
STARTS = [0, 8, 16, 24, 32, 40, 48, 54]  # per-core output-row band starts

_STATE = {}


def _build_program():
    import concourse.tile as tile
    from concourse import bacc, mybir

    bf = mybir.dt.bfloat16
    f32 = mybir.dt.float32

    nc = bacc.Bacc(None, target_bir_lowering=False)
    featA = nc.dram_tensor("featA", [128, 32, 10, 66], bf, kind="ExternalInput")
    featB = nc.dram_tensor("featB", [128, 32, 10, 66], bf, kind="ExternalInput")
    # [hg, oct, p=(d,c), wp*r*j*o] ; r in 0..3 (the K=128 ktiles)
    wmain = nc.dram_tensor("wmain", [2, 8, 128, 8192], bf, kind="ExternalInput")
    # [hg, half, p=c, wq*j*o] ; kt4 (K=64, (i,j)=(2,2))
    wkt4 = nc.dram_tensor("wkt4", [2, 2, 64, 8192], bf, kind="ExternalInput")
    outS = nc.dram_tensor("outS", [2, 128, 4096], bf, kind="ExternalOutput")

    with tile.TileContext(nc) as tc:
        with tc.tile_pool(name="feat", bufs=1) as fpool, \
             tc.tile_pool(name="wm", bufs=3) as wpool, \
             tc.tile_pool(name="k4", bufs=2) as kpool, \
             tc.tile_pool(name="st", bufs=2) as spool, \
             tc.tile_pool(name="ps", bufs=8, space="PSUM") as pspool:
            fA = fpool.tile([128, 32, 10, 66], bf)
            nc.sync.dma_start(fA[:], featA[:])
            fB = fpool.tile([128, 32, 10, 66], bf)
            nc.sync.dma_start(fB[:], featB[:])
            for hg in range(2):
                hl = 4 * hg
                S = spool.tile([128, 4096], bf)
                for octi in range(8):
                    wm = wpool.tile([128, 8192], bf)
                    nc.sync.dma_start(wm[:], wmain[hg, octi])
                    if octi % 4 == 0:
                        k4 = kpool.tile([64, 8192], bf)
                        nc.sync.dma_start(k4[:], wkt4[hg, octi // 4])
                    for wp in range(8):
                        w = 8 * octi + wp
                        ps = pspool.tile([128, 64], f32)
                        for j in range(4):
                            for r in range(5):
                                if r <= 2:
                                    lhsT = fA[:, :, hl + j + r, w]
                                elif r == 3:
                                    lhsT = fB[:, :, hl + j, w + 2]
                                else:
                                    lhsT = fA[0:64, :, hl + j + 2, w + 2]
                                if r < 4:
                                    off = ((wp * 4 + r) * 4 + j) * 64
                                    rhs = wm[:, off:off + 64]
                                else:
                                    off = ((w - 32 * (octi // 4)) * 4 + j) * 64
                                    rhs = k4[:, off:off + 64]
                                nc.tensor.matmul(
                                    ps[32 * j:32 * j + 32, :], lhsT, rhs,
                                    start=(r == 0), stop=(r == 4),
                                    tile_position=(0, 32 * j),
                                )
                        col = (octi * 8 + wp) * 64
                        nc.vector.tensor_copy(S[:, col:col + 64], ps[:])
                nc.sync.dma_start(outS[hg], S[:])
    nc.compile()
    return nc


def _get_nc():
    if "nc" not in _STATE:
        _STATE["nc"] = _build_program()
    return _STATE["nc"]


def _prep_inputs(features, weights):
    """Build the 8 per-core input dicts (all bf16, device layouts)."""
    x = np.asarray(features, dtype=np.float32)
    Wt = np.asarray(weights, dtype=np.float32)

    # (r, d) -> (i, j) selection for the K=128 ktiles
    SEL = {(0, 0): (0, 0), (0, 1): (0, 1),
           (1, 0): (1, 0), (1, 1): (1, 1),
           (2, 0): (2, 0), (2, 1): (2, 1),
           (3, 0): (0, 2), (3, 1): (1, 2)}

    in_maps = []
    for s in STARTS:
        xt = x[:, :, s:s + 10, :].transpose(1, 0, 2, 3)  # [c, b, 10, 64]
        fA = np.zeros((128, 32, 10, 66), dtype=BF16)
        fA[:64, :, :, :64] = xt
        fA[64:, :, :, :63] = xt[:, :, :, 1:]
        fB = np.zeros((128, 32, 10, 66), dtype=BF16)
        fB[:64, :, :, :64] = xt
        fB[64:, :, :9, :64] = xt[:, :, 1:, :]

        Wb = Wt[s:s + 8]                                  # [8, 62, o, c, 3, 3]
        Wpad = np.zeros((8, 64, COUT, CIN, 3, 3), dtype=np.float32)
        Wpad[:, :62] = Wb

        wmain = np.zeros((2, 8, 128, 8, 4, 4, 64), dtype=BF16)
        for (r, d), (i_s, j_s) in SEL.items():
            src = Wpad[:, :, :, :, i_s, j_s]              # [8h, 64w, o, c]
            # -> [hg, oct, c, wp, j, o]
            src2 = src.reshape(2, 4, 8, 8, COUT, CIN).transpose(0, 2, 5, 3, 1, 4)
            wmain[:, :, d * 64:(d + 1) * 64, :, r, :, :] = src2
        wmain = wmain.reshape(2, 8, 128, 8192)

        srck = Wpad[:, :, :, :, 2, 2]                     # [8h, 64w, o, c]
        # -> [hg, half, c, wq, j, o]
        wk = srck.reshape(2, 4, 2, 32, COUT, CIN).transpose(0, 2, 5, 3, 1, 4)
        wkt4 = np.ascontiguousarray(wk, dtype=BF16).reshape(2, 2, 64, 8192)

        in_maps.append({"featA": fA, "featB": fB, "wmain": wmain, "wkt4": wkt4})
    return in_maps


def _gather(results, bias):
    out = np.zeros((B, COUT, HOUT, WOUT), dtype=np.float32)
    for core, s in enumerate(STARTS):
        arr = np.asarray(results[core]["outS"]).astype(np.float32)
        # [hg, j, b, oct, wp, o] -> [b, o, hg, j, oct, wp]
        arr = arr.reshape(2, 4, 32, 8, 8, 64).transpose(2, 5, 0, 1, 3, 4)
        out[:, :, s:s + 8, :] = arr.reshape(32, 64, 8, 64)[:, :, :, :62]
    out += np.asarray(bias, dtype=np.float32).transpose(2, 0, 1)[None]
    return out


def _run(in_maps, trace=False, trace_cores=None):
    from concourse.bass_utils import run_bass_kernel_spmd
    nc = _get_nc()
    return run_bass_kernel_spmd(
        nc, in_maps, core_ids=list(range(NCORES)),
        trace=trace, trace_cores=trace_cores,
    )


def kernel(features, weights, bias):
    in_maps = _prep_inputs(features, weights)
    res = _run(in_maps)
    return _gather(res.results, bias)


# revision 6
# speedup vs baseline: 1.0515x; 1.0515x over previous
"""LocallyConnected2d kernel for 8 TRN2 NeuronCores (Bass/Tile).

Problem (hardcoded):
  features [32, 64, 64, 64] f32, weights [62, 62, 64, 64, 3, 3] f32,
  bias [62, 62, 64] f32 -> out [32, 64, 62, 62] f32
  out[b,o,h,w] = sum_{c,i,j} x[b,c,h+i,w+j] * W[h,w,o,c,i,j] + bias[h,w,o]

Strategy:
  - Shard over Hout: 8 cores x 8 output rows (bands [0,8,...,48,54], last two
    overlap by 2 rows; host takes the canonical rows from each core).
  - bf16 compute on the PE. Per output location (h,w): contraction over
    (c,i,j)=576 done as 5 accumulating matmuls into PSUM:
      kt r in {0,1,2}: K=128 = [c x (i=r,j=0) | c x (i=r,j=1)]
      kt 3:            K=128 = [c x (0,2)     | c x (1,2)]
      kt 4:            K=64  =  c x (2,2)
    The K=128 tiles read a host-baked "dual shifted" feature layout:
      featA[p<64]  = x[c, t, w]      featA[p>=64] = x[c, t, w+1]
      featB[p<64]  = x[c, t, w]      featB[p>=64] = x[c, t+1, w]
    so a single [128, 32] AP slice is a ready im2col patch (batch = free dim).
  - Patches are the stationary operand (cols = batch, 32); weights stream as
    the moving operand [K, 64]. 4 output rows (j=0..3) are packed into one
    PSUM tile [128, 64] via col tile_position=(0, 32j).
  - PSUM -> SBUF (bf16 cast) -> one contiguous DMA per half-band; host
    unpacks, adds bias, and assembles the full f32 output.
"""

import numpy as np
import ml_dtypes

BF16 = ml_dtypes.bfloat16

B, CIN, COUT = 32, 64, 64
H = W = 64
HOUT = WOUT = 62
NCORES = 8
STARTS = [0, 8, 16, 24, 32, 40, 48, 54]  # per-core output-row band starts

_STATE = {}


def _build_program():
    import concourse.tile as tile
    from concourse import bacc, mybir

    bf = mybir.dt.bfloat16
    f32 = mybir.dt.float32

    nc = bacc.Bacc(None, target_bir_lowering=False)
    featA = nc.dram_tensor("featA", [128, 10, 66, 32], bf, kind="ExternalInput")
    featB = nc.dram_tensor("featB", [128, 10, 66, 32], bf, kind="ExternalInput")
    # [hg, oct, p=(d,c), wp*r*j*o] ; r in 0..3 (the K=128 ktiles)
    wmain = nc.dram_tensor("wmain", [2, 8, 128, 8192], bf, kind="ExternalInput")
    # [hg, half, p=c, wq*j*o] ; kt4 (K=64, (i,j)=(2,2))
    wkt4 = nc.dram_tensor("wkt4", [2, 2, 64, 8192], bf, kind="ExternalInput")
    outS = nc.dram_tensor("outS", [2, 128, 4096], bf, kind="ExternalOutput")

    with tile.TileContext(nc) as tc:
        with tc.tile_pool(name="feat", bufs=1) as fpool, \
             tc.tile_pool(name="wm", bufs=4) as wpool, \
             tc.tile_pool(name="k4", bufs=2) as kpool, \
             tc.tile_pool(name="st", bufs=2) as spool, \
             tc.tile_pool(name="ps", bufs=8, space="PSUM") as pspool:
            # features + output store ride the ACT HWDGE ring; the weight
            # stream rides the SP ring -> no head-of-line blocking between them
            fA = fpool.tile([128, 10, 66, 32], bf)
            nc.scalar.dma_start(fA[:], featA[:])
            fB = fpool.tile([128, 10, 66, 32], bf)
            nc.scalar.dma_start(fB[:], featB[:])
            for hg in range(2):
                hl = 4 * hg
                S = spool.tile([128, 4096], bf)
                for octi in range(8):
                    wm = wpool.tile([128, 8192], bf)
                    nc.sync.dma_start(wm[:], wmain[hg, octi])
                    if octi % 4 == 0:
                        k4 = kpool.tile([64, 8192], bf)
                        nc.sync.dma_start(k4[:], wkt4[hg, octi // 4])
                    for wp in range(8):
                        w = 8 * octi + wp
                        ps = pspool.tile([128, 64], f32)
                        for j in range(4):
                            for r in range(5):
                                if r <= 2:
                                    lhsT = fA[:, hl + j + r, w, :]
                                elif r == 3:
                                    lhsT = fB[:, hl + j, w + 2, :]
                                else:
                                    lhsT = fA[0:64, hl + j + 2, w + 2, :]
                                if r < 4:
                                    off = ((wp * 4 + r) * 4 + j) * 64
                                    rhs = wm[:, off:off + 64]
                                else:
                                    off = ((w - 32 * (octi // 4)) * 4 + j) * 64
                                    rhs = k4[:, off:off + 64]
                                nc.tensor.matmul(
                                    ps[32 * j:32 * j + 32, :], lhsT, rhs,
                                    start=(r == 0), stop=(r == 4),
                                    tile_position=(0, 32 * j),
                                )
                        col = (octi * 8 + wp) * 64
                        nc.vector.tensor_copy(S[:, col:col + 64], ps[:])
                nc.scalar.dma_start(outS[hg], S[:])
    nc.compile()
    return nc


def _get_nc():
    if "nc" not in _STATE:
        _STATE["nc"] = _build_program()
    return _STATE["nc"]


def _prep_inputs(features, weights):
    """Build the 8 per-core input dicts (all bf16, device layouts)."""
    x = np.asarray(features, dtype=np.float32)
    Wt = np.asarray(weights, dtype=np.float32)

    # (r, d) -> (i, j) selection for the K=128 ktiles
    SEL = {(0, 0): (0, 0), (0, 1): (0, 1),
           (1, 0): (1, 0), (1, 1): (1, 1),
           (2, 0): (2, 0), (2, 1): (2, 1),
           (3, 0): (0, 2), (3, 1): (1, 2)}

    in_maps = []
    for s in STARTS:
        xt = x[:, :, s:s + 10, :].transpose(1, 2, 3, 0)  # [c, 10, 64, b]
        fA = np.zeros((128, 10, 66, 32), dtype=BF16)
        fA[:64, :, :64, :] = xt
        fA[64:, :, :63, :] = xt[:, :, 1:, :]
        fB = np.zeros((128, 10, 66, 32), dtype=BF16)
        fB[:64, :, :64, :] = xt
        fB[64:, :9, :64, :] = xt[:, 1:, :, :]

        Wb = Wt[s:s + 8]                                  # [8, 62, o, c, 3, 3]
        Wpad = np.zeros((8, 64, COUT, CIN, 3, 3), dtype=np.float32)
        Wpad[:, :62] = Wb

        wmain = np.zeros((2, 8, 128, 8, 4, 4, 64), dtype=BF16)
        for (r, d), (i_s, j_s) in SEL.items():
            src = Wpad[:, :, :, :, i_s, j_s]              # [8h, 64w, o, c]
            # -> [hg, oct, c, wp, j, o]
            src2 = src.reshape(2, 4, 8, 8, COUT, CIN).transpose(0, 2, 5, 3, 1, 4)
            wmain[:, :, d * 64:(d + 1) * 64, :, r, :, :] = src2
        wmain = wmain.reshape(2, 8, 128, 8192)

        srck = Wpad[:, :, :, :, 2, 2]                     # [8h, 64w, o, c]
        # -> [hg, half, c, wq, j, o]
        wk = srck.reshape(2, 4, 2, 32, COUT, CIN).transpose(0, 2, 5, 3, 1, 4)
        wkt4 = np.ascontiguousarray(wk, dtype=BF16).reshape(2, 2, 64, 8192)

        in_maps.append({"featA": fA, "featB": fB, "wmain": wmain, "wkt4": wkt4})
    return in_maps


def _gather(results, bias):
    out = np.zeros((B, COUT, HOUT, WOUT), dtype=np.float32)
    for core, s in enumerate(STARTS):
        arr = np.asarray(results[core]["outS"]).astype(np.float32)
        # [hg, j, b, oct, wp, o] -> [b, o, hg, j, oct, wp]
        arr = arr.reshape(2, 4, 32, 8, 8, 64).transpose(2, 5, 0, 1, 3, 4)
        out[:, :, s:s + 8, :] = arr.reshape(32, 64, 8, 64)[:, :, :, :62]
    out += np.asarray(bias, dtype=np.float32).transpose(2, 0, 1)[None]
    return out


def _run(in_maps, trace=False, trace_cores=None):
    from concourse.bass_utils import run_bass_kernel_spmd
    nc = _get_nc()
    return run_bass_kernel_spmd(
        nc, in_maps, core_ids=list(range(NCORES)),
        trace=trace, trace_cores=trace_cores,
    )


def kernel(features, weights, bias):
    in_maps = _prep_inputs(features, weights)
    res = _run(in_maps)
    return _gather(res.results, bias)


# revision 9
# speedup vs baseline: 1.5679x; 1.4911x over previous
"""LocallyConnected2d kernel for 8 TRN2 NeuronCores (Bass/Tile).

Problem (hardcoded):
  features [32, 64, 64, 64] f32, weights [62, 62, 64, 64, 3, 3] f32,
  bias [62, 62, 64] f32 -> out [32, 64, 62, 62] f32
  out[b,o,h,w] = sum_{c,i,j} x[b,c,h+i,w+j] * W[h,w,o,c,i,j] + bias[h,w,o]

Strategy:
  - Shard over Hout: 8 cores x 8 output rows (bands [0,8,...,48,54], the last
    two overlap; host takes canonical rows from each core).
  - bf16 on the PE, fp32 PSUM accumulate. Contraction (c,i,j)=576 per output
    location via 14 matmuls per location-group, built on a host-baked
    "dual shifted" feature layout (partition p<64: x[c,t,w]; p>=64 carries a
    shifted copy) so a [128,32] AP slice is a ready im2col patch
    (batch = stationary cols).
  - Work unit = (half-band hg, group of 4 w): PSUM tile [128,256] with
    partitions=(4w x 32b) via col tile_position and free=(4 output rows x 64
    cout). ONE accumulation group per tile (single start=True; per-element
    has_written gives overwrite-on-first-touch) -> no mid-tile start stalls.
  - Matmuls grouped by stationary: a patch at absolute row t serves all
    (out-row j, kernel-row r) with j+r=t-hl in ONE matmul with a wide moving
    operand (weights host-concatenated, N up to 192).
  - DMA spread over both HWDGE rings (sync/scalar) + SWDGE (gpsimd).
  - Host: shard/pack inputs, unpack outS dumps, add bias, assemble f32 out.
"""

import numpy as np
import ml_dtypes

BF16 = ml_dtypes.bfloat16

B, CIN, COUT = 32, 64, 64
H = W = 64
HOUT = WOUT = 62
NCORES = 8
STARTS = [0, 8, 16, 24, 32, 40, 48, 54]

# t-group geometry: tau = t - hl in 0..5; valid out-rows j in [jlo, jhi]
TAUS = list(range(6))
JLO = [max(0, t - 2) for t in TAUS]
JHI = [min(3, t) for t in TAUS]
NV = [hi - lo + 1 for lo, hi in zip(JLO, JHI)]          # [1,2,3,3,2,1]
TBASE = [0]
for t in TAUS:
    TBASE.append(TBASE[-1] + 4 * NV[t] * 64)            # per-(tau) base col
WR_COLS = TBASE[-1]                                      # 3072

_STATE = {}


def _build_program():
    import concourse.tile as tile
    from concourse import bacc, mybir

    bf = mybir.dt.bfloat16
    f32 = mybir.dt.float32

    nc = bacc.Bacc(None, target_bir_lowering=False)
    featA = nc.dram_tensor("featA", [128, 10, 66, 32], bf, kind="ExternalInput")
    featB = nc.dram_tensor("featB", [128, 10, 66, 32], bf, kind="ExternalInput")
    wr_d = nc.dram_tensor("wr", [2, 16, 128, WR_COLS], bf, kind="ExternalInput")
    w3_d = nc.dram_tensor("w3", [2, 16, 128, 1024], bf, kind="ExternalInput")
    w4_d = nc.dram_tensor("w4", [2, 16, 64, 1024], bf, kind="ExternalInput")
    outS = nc.dram_tensor("outS", [2, 128, 4096], bf, kind="ExternalOutput")

    with tile.TileContext(nc) as tc:
        with tc.tile_pool(name="feat", bufs=1) as fpool, \
             tc.tile_pool(name="wr", bufs=4) as wrpool, \
             tc.tile_pool(name="w3", bufs=4) as w3pool, \
             tc.tile_pool(name="w4", bufs=4) as w4pool, \
             tc.tile_pool(name="st", bufs=2) as spool, \
             tc.tile_pool(name="ps", bufs=8, space="PSUM") as pspool:
            # features: halves ride different rings (disjoint SDMA engine sets)
            fA = fpool.tile([128, 10, 66, 32], bf)
            nc.sync.dma_start(fA[0:64], featA[0:64])
            nc.scalar.dma_start(fA[64:128], featA[64:128])
            fB = fpool.tile([128, 10, 66, 32], bf)
            nc.gpsimd.dma_start(fB[0:64], featB[0:64])
            nc.scalar.dma_start(fB[64:128], featB[64:128])
            # zero operands for the psum-clearing matmul (see below)
            zl = fpool.tile([1, 128], bf)
            nc.gpsimd.memset(zl[:], 0.0)
            zr = fpool.tile([1, 256], bf)
            nc.gpsimd.memset(zr[:], 0.0)
            for hg in range(2):
                hl = 4 * hg
                S = spool.tile([128, 4096], bf)
                for wg in range(16):
                    w0 = 4 * wg
                    eng = nc.sync if wg % 2 == 0 else nc.scalar
                    eng2 = nc.scalar if wg % 2 == 0 else nc.sync
                    wr = wrpool.tile([128, WR_COLS], bf)
                    eng.dma_start(wr[:], wr_d[hg, wg])
                    w3 = w3pool.tile([128, 1024], bf)
                    eng2.dma_start(w3[:], w3_d[hg, wg])
                    w4 = w4pool.tile([64, 1024], bf)
                    nc.gpsimd.dma_start(w4[:], w4_d[hg, wg])

                    ps = pspool.tile([128, 256], f32)
                    # K=1 zeroing matmul over the WHOLE tile: starts the
                    # accumulation group, zeroes every element, and (because
                    # its output overlaps all later MMs) forces the scheduler
                    # to keep it first; all real MMs are then pure order-free
                    # flags=0 accumulates.
                    nc.tensor.matmul(ps[:, :], zl[:], zr[:],
                                     start=True, stop=False,
                                     tile_position=(0, 0))
                    for tau in TAUS:
                        nv, jlo = NV[tau], JLO[tau]
                        for g in range(4):
                            off = TBASE[tau] + g * nv * 64
                            nc.tensor.matmul(
                                ps[32 * g:32 * g + 32, 64 * jlo:64 * (jlo + nv)],
                                fA[:, hl + tau, w0 + g, :],
                                wr[:, off:off + nv * 64],
                                start=False, stop=False,
                                tile_position=(0, 32 * g),
                            )
                    for j in range(4):
                        for g in range(4):
                            off = (j * 4 + g) * 64
                            nc.tensor.matmul(
                                ps[32 * g:32 * g + 32, 64 * j:64 * j + 64],
                                fB[:, hl + j, w0 + g + 2, :],
                                w3[:, off:off + 64],
                                start=False, stop=False,
                                tile_position=(0, 32 * g),
                            )
                    for j in range(4):
                        for g in range(4):
                            off = (j * 4 + g) * 64
                            nc.tensor.matmul(
                                ps[32 * g:32 * g + 32, 64 * j:64 * j + 64],
                                fA[0:64, hl + j + 2, w0 + g + 2, :],
                                w4[:, off:off + 64],
                                start=False, stop=(j == 3 and g == 3),
                                tile_position=(0, 32 * g),
                            )
                    nc.vector.tensor_copy(S[:, 256 * wg:256 * wg + 256], ps[:])
                nc.gpsimd.dma_start(outS[hg], S[:])
    nc.compile()
    return nc


def _get_nc():
    if "nc" not in _STATE:
        _STATE["nc"] = _build_program()
    return _STATE["nc"]


def _prep_inputs(features, weights):
    """Build the 8 per-core input dicts (bf16, device layouts)."""
    x = np.asarray(features, dtype=np.float32)
    Wt = np.asarray(weights, dtype=np.float32)

    in_maps = []
    for s in STARTS:
        xt = x[:, :, s:s + 10, :].transpose(1, 2, 3, 0)  # [c, 10, 64, b]
        fA = np.zeros((128, 10, 66, 32), dtype=BF16)
        fA[:64, :, :64, :] = xt
        fA[64:, :, :63, :] = xt[:, :, 1:, :]             # w+1 shift
        fB = np.zeros((128, 10, 66, 32), dtype=BF16)
        fB[:64, :, :64, :] = xt
        fB[64:, :9, :64, :] = xt[:, 1:, :, :]            # h+1 shift

        Wb = Wt[s:s + 8]                                  # [8, 62, o, c, 3, 3]
        Wpad = np.zeros((8, 64, COUT, CIN, 3, 3), dtype=np.float32)
        Wpad[:, :62] = Wb
        WT = Wpad.transpose(4, 5, 3, 0, 1, 2)             # [i, jw, c, 8h, 64w, o]

        # wr: t-grouped ktiles (cells (r,0)|(r,1)); cols per (tau,g):
        #   q=0..nv-1 -> j=jlo+q, r=tau-j; value(d,c,o)=W[h,w,o,c,r,d]
        wr = np.zeros((2, 16, 128, WR_COLS), dtype=BF16)
        for tau in TAUS:
            nv, jlo = NV[tau], JLO[tau]
            view = wr[:, :, :, TBASE[tau]:TBASE[tau + 1]].reshape(
                2, 16, 128, 4, nv, 64)
            for q in range(nv):
                j = jlo + q
                r = tau - j
                for d in range(2):
                    src = WT[r, d].reshape(CIN, 2, 4, 16, 4, COUT)[:, :, j]
                    view[:, :, d * 64:(d + 1) * 64, :, q, :] = \
                        src.transpose(1, 2, 0, 3, 4)      # [hg, wg, c, g, o]
        # w3: cells (0,2) d=0 / (1,2) d=1 ; free=(j,g,o)
        w3 = np.zeros((2, 16, 128, 1024), dtype=BF16)
        for d in range(2):
            src = WT[d, 2].reshape(CIN, 2, 4, 16, 4, COUT)
            w3[:, :, d * 64:(d + 1) * 64, :] = src.transpose(
                1, 3, 0, 2, 4, 5).reshape(2, 16, 64, 1024)
        # w4: cell (2,2)
        src = WT[2, 2].reshape(CIN, 2, 4, 16, 4, COUT)
        w4 = np.ascontiguousarray(
            src.transpose(1, 3, 0, 2, 4, 5), dtype=BF16).reshape(2, 16, 64, 1024)

        in_maps.append({"featA": fA, "featB": fB, "wr": wr, "w3": w3, "w4": w4})
    return in_maps


def _gather(results, bias):
    out = np.zeros((B, COUT, HOUT, WOUT), dtype=np.float32)
    for core, s in enumerate(STARTS):
        arr = np.asarray(results[core]["outS"]).astype(np.float32)
        # [hg, g, b, wg, j, o] -> [b, o, hg, j, wg, g]
        arr = arr.reshape(2, 4, 32, 16, 4, 64).transpose(2, 5, 0, 4, 3, 1)
        out[:, :, s:s + 8, :] = arr.reshape(32, 64, 8, 64)[:, :, :, :62]
    out += np.asarray(bias, dtype=np.float32).transpose(2, 0, 1)[None]
    return out


def _run(in_maps, trace=False, trace_cores=None):
    from concourse.bass_utils import run_bass_kernel_spmd
    nc = _get_nc()
    return run_bass_kernel_spmd(
        nc, in_maps, core_ids=list(range(NCORES)),
        trace=trace, trace_cores=trace_cores,
    )


def kernel(features, weights, bias):
    in_maps = _prep_inputs(features, weights)
    res = _run(in_maps)
    return _gather(res.results, bias)
